# revision 6
# baseline (speedup 1.0000x reference)
"""GQA attention layer (QKV proj + RoPE + softmax attention + out proj) on 8
Trainium2 NeuronCores.

Sharding: core c = (batch b, head-group g) with b = c // 4, g = c % 4.
Each core handles one batch and one GQA group (4 q heads + 1 kv head),
computes a w_o-column-sliced partial output (row-parallel), and the host sums
the 4 partials per batch.

v2 redesign (cost-model driven):
- PV matmul flipped to [q-chunk, 65]-shaped output tiles accumulated over k
  chunks: the PE charge per accumulation step drops from N=512 to N=65, and
  softmax normalization becomes a cheap per-partition tensor_scalar instead
  of fp32 broadcast matmuls (4 cyc/row).
- Denominator rides along as the 65th 'ones' column of the V-augmented tile;
  only the first/last group per 2KB PSUM zero-region carries start/stop so
  four accumulation groups can share a bank.
- Normalized [q,d] tiles are transposed back to [d,q] on the PE for the
  output projection.
- The exp stream on the Activation engine (~134us busy) is the critical
  path: everything else hides behind it.  The per-unit softmax finalize is
  split into a DVE-only normalize (emitted after the next unit's first two
  score/exp pairs, so psA turns around without stalling Act) and
  transpose/copy work that drains through the filler queue.
- RoPE: q uses a rotate-projection (the rotate-half permutation folded into
  a second, host-permuted weight matrix) so the rotation is three full-width
  DVE ops off a PSUM read; k uses the classic 4-op form reading PSUM
  directly (PSUM-input elementwise ops may mix partition bases; SBUF/SBUF
  may not, per the BIR verifier).
- V is projected in flipped [token, d] orientation straight into the
  V-augmented tile, eliminating the PE transpose round trip.
- Startup streams inputs over three DMA queues (SP / Activation / Pool
  SWDGE); the tail interleaves the last unit's normalize per q-chunk with
  its output projection across four engines.
"""

import numpy as np
import ml_dtypes

B, S, HID = 2, 2048, 1024
NH, NKV, D = 16, 4, 64
SCALE = D ** -0.5
NCORES = 8
TT = 512          # token tile (projection N / q tile)
NTT = S // TT     # 4
KC = S // 128     # 16 k chunks

_BF16 = ml_dtypes.bfloat16

_nc_cache = None


def _build_bass():
    import concourse.bass as bass
    import concourse.mybir as mybir
    import concourse.tile as tile
    from concourse import bacc
    from concourse.masks import make_identity

    BF = mybir.dt.bfloat16
    F32 = mybir.dt.float32
    AF = mybir.ActivationFunctionType
    MULT = mybir.AluOpType.mult
    ADD = mybir.AluOpType.add

    nc = bacc.Bacc()
    hT = nc.dram_tensor("hT", (HID, S), BF, kind="ExternalInput")
    wqkT = nc.dram_tensor("wqkT", (HID, 384), BF, kind="ExternalInput")
    woT = nc.dram_tensor("woT", (256, HID), BF, kind="ExternalInput")
    cos2d = nc.dram_tensor("cos2d", (128, S), BF, kind="ExternalInput")
    sin2d = nc.dram_tensor("sin2d", (128, S), BF, kind="ExternalInput")
    sind = nc.dram_tensor("sind", (64, S), BF, kind="ExternalInput")
    wqrd = nc.dram_tensor("wqrd", (HID, 320), BF, kind="ExternalInput")
    out = nc.dram_tensor("out", (S, HID), F32, kind="ExternalOutput")

    # psA group layout: 8 groups of [64 attn | 1 den] columns; keep each
    # group's 65 columns inside one 2KB PSUM bank (g<4 bank0, g>=4 bank1)
    GOFF = [65 * g if g < 4 else 512 + 65 * (g - 4) for g in range(8)]

    with tile.TileContext(nc) as tc:
        with (
            tc.tile_pool(name="persist", bufs=1) as pp,
            tc.tile_pool(name="rope", bufs=3) as rp,
            tc.tile_pool(name="exps", bufs=8) as ep,
            tc.tile_pool(name="norm", bufs=8) as np_,
            tc.tile_pool(name="outsb", bufs=4) as op_,
        ):
            # ---- persistent SBUF tiles + input loads (chunked for DMA spread)
            h_sb = pp.tile([128, 8, S], BF, tag="h_sb")
            wqk_sb = pp.tile([128, 8, 384], BF, tag="wqk_sb")
            wqr_sb = pp.tile([128, 8, 320], BF, tag="wqr_sb")
            wo_sb = pp.tile([128, 2, HID], BF, tag="wo_sb")
            # cos/sin duplicated across both partition halves (full-width
            # rot-projection RoPE); sin_sb is the sign-folded classic table
            cos2_sb = pp.tile([128, S], BF, tag="cos2_sb")
            sin2_sb = pp.tile([128, S], BF, tag="sin2_sb")
            cos_sb = cos2_sb[0:64, :]
            sin_sb = pp.tile([64, S], BF, tag="sin_sb")
            h_dram = hT.rearrange("(c p) s -> p c s", p=128)
            wqk_dram = wqkT.rearrange("(c p) r -> p c r", p=128)
            wqr_dram = wqrd.rearrange("(c p) r -> p c r", p=128)
            wo_dram = woT.rearrange("(c p) h -> p c h", p=128)
            # three-queue head: h chunks stream on SP (500ns/issue), wqk
            # splits across the Activation HWDGE queue (after the first
            # cos/sin quarter that RoPE tt0 needs) and the Pool SWDGE queue,
            # so the first rc2 projection is fed from ~1.5us on
            nc.sync.dma_start(wqk_sb[:, 0, :], wqk_dram[:, 0, :])
            nc.sync.dma_start(wqk_sb[:, 1, :], wqk_dram[:, 1, :])
            for hc in range(2, 4):
                nc.scalar.dma_start(wqk_sb[:, hc, :], wqk_dram[:, hc, :])
            nc.scalar.dma_start(cos2_sb[:, 0:TT], cos2d[:, 0:TT])
            nc.scalar.dma_start(sin2_sb[:, 0:TT], sin2d[:, 0:TT])
            nc.scalar.dma_start(sin_sb[:, 0:TT], sind[:, 0:TT])
            nc.scalar.dma_start(wqr_sb[:, :, 256:320],
                                wqr_dram[:, :, 256:320])
            nc.scalar.dma_start(wqr_sb[:, :, 0:128],
                                wqr_dram[:, :, 0:128])
            for hc in range(4, 8):
                nc.gpsimd.dma_start(wqk_sb[:, hc, :], wqk_dram[:, hc, :])
            for hc in range(8):
                nc.sync.dma_start(h_sb[:, hc, 0:TT], h_dram[:, hc, 0:TT])
            for tt in range(1, NTT):
                for hc in range(8):
                    tts_ = bass.ts(tt, TT)
                    nc.sync.dma_start(h_sb[:, hc, tts_], h_dram[:, hc, tts_])
            # second halves of the RoPE tables ride the idle Pool SWDGE
            # queue (land ~10us, needed from ~13us on)
            nc.gpsimd.dma_start(cos2_sb[:, TT:S], cos2d[:, TT:S])
            nc.gpsimd.dma_start(sin2_sb[:, TT:S], sin2d[:, TT:S])
            nc.gpsimd.dma_start(sin_sb[:, TT:S], sind[:, TT:S])
            nc.gpsimd.dma_start(wqr_sb[:, :, 128:256],
                                wqr_dram[:, :, 128:256])
            for oc in range(2):
                nc.sync.dma_start(wo_sb[:, oc, :], wo_dram[:, oc, :])

            ident = pp.tile([128, 128], BF, tag="ident")
            make_identity(nc, ident[:])
            # preload the exp table set while input DMAs stream
            warm = pp.tile([1, 8], F32, tag="warm")
            nc.any.memset(warm[:], 0.0)
            nc.scalar.activation(warm[:], warm[:], AF.Exp)

            # roped q, 2 heads per tile (head 2p at rows 0:64, 2p+1 at 64:128)
            qrot = [pp.tile([128, S], BF, tag=f"qrot{p}", name=f"qrot{p}") for p in range(2)]
            # roped k duplicated on both partition halves (matmul requires
            # lhsT/rhs base partitions to match; BIR forbids cross-base
            # SBUF/SBUF elementwise reads, so the halves are materialized)
            k2 = pp.tile([128, S], BF, tag="k2")
            # V with ones column for the fused softmax denominator
            vaug = pp.tile([128, KC, 65], BF, tag="vaug")
            nc.any.memset(vaug[:], 1.0)
            # normalized attention output in [d, token] layout per pair
            anorm = [pp.tile([128, S], BF, tag=f"anorm{o}", name=f"anorm{o}") for o in range(2)]
            # per-unit reciprocal denominators (unit = pair + 2*qt)
            rec_sb = pp.tile([128, 8, 8], F32, tag="rec_sb")

            def rope64(ps_blk, dest, tts, tmp_tag):
                """RoPE one 64-row head block [64, TT] read directly from
                PSUM (PSUM-input elementwise ops may mix partition bases;
                SBUF/SBUF may not). The bf16 t1+rt add runs in DVE 2x."""
                t1 = rp.tile([64, TT], BF, tag=f"t1{tmp_tag}")
                rt = rp.tile([64, TT], BF, tag=f"rt{tmp_tag}")
                nc.vector.tensor_tensor(t1[:], ps_blk[0:64, :], cos_sb[:, tts], MULT)
                nc.vector.tensor_tensor(
                    rt[0:32, :], ps_blk[32:64, :], sin_sb[0:32, tts], MULT
                )
                nc.vector.tensor_tensor(
                    rt[32:64, :], ps_blk[0:32, :], sin_sb[32:64, tts], MULT
                )
                nc.vector.tensor_tensor(dest, t1[:], rt[:], ADD)

            with (
                tc.tile_pool(name="psP", bufs=2, space="PSUM") as psP,
                tc.tile_pool(name="psS", bufs=2, space="PSUM") as psS,
                tc.tile_pool(name="psACC", bufs=1, space="PSUM") as psA,
            ):
                def proj_finish(rc, tt, ps, on_act=False):
                    tts = bass.ts(tt, TT)
                    if rc == 2:
                        rope64(ps[0:64, :], k2[0:64, tts], tts, "k")
                        nc.gpsimd.tensor_copy(k2[64:128, tts], k2[0:64, tts])
                    else:
                        rope64(ps[0:64, :], qrot[rc][0:64, tts], tts, "qa")
                        rope64(ps[64:128, :], qrot[rc][64:128, tts], tts, "qb")

                def wqk_cols(rc):
                    # q pairs at 0:128 / 128:256; k-only at 256:320
                    return slice(256, 320) if rc == 2 else bass.ts(rc, 128)

                def proj_chunk(rc, tt):
                    """One projection tile: 8 accumulating matmuls, then RoPE
                    (rc<2: q pair [128,TT]; rc==2: k rows only [64,TT])."""
                    tts = bass.ts(tt, TT)
                    rows = 64 if rc == 2 else 128
                    ps = psP.tile([rows, TT], F32, tag="proj",
                                  name=f"proj{rc}_{tt}")
                    for hc in range(8):
                        nc.tensor.matmul(
                            ps[:],
                            wqk_sb[:, hc, wqk_cols(rc)],
                            h_sb[:, hc, tts],
                            start=(hc == 0),
                            stop=(hc == 7),
                        )
                    proj_finish(rc, tt, ps)

                def kproj_rot(tt):
                    """k projection + rotate-half projection; k2 rows 0:64
                    via three full-width DVE ops, then the partition-half
                    duplicate for the head-B score lhsT."""
                    tts = bass.ts(tt, TT)
                    ps = psP.tile([64, TT], F32, tag="proj", name=f"kp{tt}")
                    ps2 = psP.tile([64, TT], F32, tag="proj", name=f"kr{tt}")
                    for hc in range(8):
                        nc.tensor.matmul(
                            ps[:], wqk_sb[:, hc, 256:320], h_sb[:, hc, tts],
                            start=(hc == 0), stop=(hc == 7),
                        )
                    for hc in range(8):
                        nc.tensor.matmul(
                            ps2[:], wqr_sb[:, hc, 256:320], h_sb[:, hc, tts],
                            start=(hc == 0), stop=(hc == 7),
                        )
                    t1 = rp.tile([64, TT], BF, tag="kt1")
                    t2 = rp.tile([64, TT], BF, tag="kt2")
                    nc.vector.tensor_tensor(t1[:], ps[:], cos2_sb[0:64, tts], MULT)
                    nc.vector.tensor_tensor(t2[:], ps2[:], sin2_sb[0:64, tts], MULT)
                    nc.vector.tensor_tensor(k2[0:64, tts], t1[:], t2[:], ADD)
                    nc.gpsimd.tensor_copy(k2[64:128, tts], k2[0:64, tts])

                def add_kproj_rot_filler(tt):
                    tts = bass.ts(tt, TT)
                    cell = {}

                    def mk(which, h0):
                        def go():
                            if which == 0 and h0 == 0:
                                cell["ps"] = psP.tile(
                                    [64, TT], F32, tag="proj", name=f"kp{tt}")
                                cell["ps2"] = psP.tile(
                                    [64, TT], F32, tag="proj", name=f"kr{tt}")
                            w = wqk_sb if which == 0 else wqr_sb
                            ps = cell["ps"] if which == 0 else cell["ps2"]
                            for hc in (h0, h0 + 1):
                                nc.tensor.matmul(
                                    ps[:], w[:, hc, 256:320],
                                    h_sb[:, hc, tts],
                                    start=(hc == 0), stop=(hc == 7),
                                )
                            if which == 1 and h0 == 6:
                                t1 = rp.tile([64, TT], BF, tag="kt1")
                                t2 = rp.tile([64, TT], BF, tag="kt2")
                                nc.vector.tensor_tensor(
                                    t1[:], cell["ps"][:],
                                    cos2_sb[0:64, tts], MULT)
                                nc.vector.tensor_tensor(
                                    t2[:], cell["ps2"][:],
                                    sin2_sb[0:64, tts], MULT)
                                nc.vector.tensor_tensor(
                                    k2[0:64, tts], t1[:], t2[:], ADD)
                                nc.gpsimd.tensor_copy(
                                    k2[64:128, tts], k2[0:64, tts])
                        return go

                    for which in (0, 1):
                        for h0 in (0, 2, 4, 6):
                            filler.append(mk(which, h0))

                def qproj_rot(rc, tt):
                    """q projection with the rotate-half permutation folded
                    into a second (host-permuted) weight matrix: qrot =
                    q*cos + rot(q)*sin in three full-width DVE ops."""
                    tts = bass.ts(tt, TT)
                    ps = psP.tile([128, TT], F32, tag="proj",
                                  name=f"qp{rc}_{tt}")
                    ps2 = psP.tile([128, TT], F32, tag="proj",
                                   name=f"qr{rc}_{tt}")
                    for hc in range(8):
                        nc.tensor.matmul(
                            ps[:], wqk_sb[:, hc, bass.ts(rc, 128)],
                            h_sb[:, hc, tts],
                            start=(hc == 0), stop=(hc == 7),
                        )
                    for hc in range(8):
                        nc.tensor.matmul(
                            ps2[:], wqr_sb[:, hc, bass.ts(rc, 128)],
                            h_sb[:, hc, tts],
                            start=(hc == 0), stop=(hc == 7),
                        )
                    t1 = rp.tile([128, TT], BF, tag="rt1")
                    t2 = rp.tile([128, TT], BF, tag="rt2")
                    nc.vector.tensor_tensor(t1[:], ps[:], cos2_sb[:, tts], MULT)
                    nc.vector.tensor_tensor(t2[:], ps2[:], sin2_sb[:, tts], MULT)
                    nc.vector.tensor_tensor(qrot[rc][:, tts], t1[:], t2[:], ADD)

                def add_qproj_rot_filler(rc, tt):
                    tts = bass.ts(tt, TT)
                    cell = {}

                    def mk(which, h0):
                        def go():
                            if which == 0 and h0 == 0:
                                cell["ps"] = psP.tile(
                                    [128, TT], F32, tag="proj",
                                    name=f"qp{rc}_{tt}")
                                cell["ps2"] = psP.tile(
                                    [128, TT], F32, tag="proj",
                                    name=f"qr{rc}_{tt}")
                            w = wqk_sb if which == 0 else wqr_sb
                            ps = cell["ps"] if which == 0 else cell["ps2"]
                            for hc in (h0, h0 + 1):
                                nc.tensor.matmul(
                                    ps[:], w[:, hc, bass.ts(rc, 128)],
                                    h_sb[:, hc, tts],
                                    start=(hc == 0), stop=(hc == 7),
                                )
                            if which == 1 and h0 == 6:
                                t1 = rp.tile([128, TT], BF, tag="rt1")
                                t2 = rp.tile([128, TT], BF, tag="rt2")
                                nc.vector.tensor_tensor(
                                    t1[:], cell["ps"][:], cos2_sb[:, tts], MULT)
                                nc.vector.tensor_tensor(
                                    t2[:], cell["ps2"][:], sin2_sb[:, tts], MULT)
                                nc.vector.tensor_tensor(
                                    qrot[rc][:, tts], t1[:], t2[:], ADD)
                        return go

                    for which in (0, 1):
                        for h0 in (0, 2, 4, 6):
                            filler.append(mk(which, h0))

                def vproj_one(c):
                    # V projection flipped to [token, d] orientation: writes
                    # the vaug [kpos, d] layout directly (no PE transpose)
                    tcs = bass.ts(c, 128)
                    vp = psP.tile([128, 64], F32, tag="proj", name=f"vp{c}")
                    for hc in range(8):
                        nc.tensor.matmul(
                            vp[:],
                            h_sb[:, hc, tcs],
                            wqk_sb[:, hc, 320:384],
                            start=(hc == 0),
                            stop=(hc == 7),
                        )
                    nc.vector.tensor_copy(vaug[:, c, 0:64], vp[:])

                # filler queue: zero-arg closures, each emitting ~1-2 PE
                # matmuls (plus attached vector/pool work)
                filler = []

                def emit_filler(n):
                    for _ in range(n):
                        if not filler:
                            return
                        filler.pop(0)()

                def add_proj_filler(rc, tt):
                    # split one proj chunk into 4 closures of 2 matmuls each
                    tts = bass.ts(tt, TT)
                    cell = {}

                    def mk(h0):
                        def go():
                            if h0 == 0:
                                cell["ps"] = psP.tile(
                                    [64 if rc == 2 else 128, TT], F32,
                                    tag="proj", name=f"proj{rc}_{tt}",
                                )
                            ps = cell["ps"]
                            for hc in (h0, h0 + 1):
                                nc.tensor.matmul(
                                    ps[:],
                                    wqk_sb[:, hc, wqk_cols(rc)],
                                    h_sb[:, hc, tts],
                                    start=(hc == 0),
                                    stop=(hc == 7),
                                )
                            if h0 == 6:
                                proj_finish(rc, tt, ps)
                        return go

                    for h0 in (0, 2, 4, 6):
                        filler.append(mk(h0))

                def add_vproj_filler(tt):
                    for c in range(4 * tt, 4 * tt + 4):
                        filler.append(lambda c=c: vproj_one(c))

                def outproj_one(tch, ht, ob_on_act=False, dma_q="sp",
                                pool=None, ptag="proj"):
                    tcs = bass.ts(tch, 128)
                    hts = bass.ts(ht, TT)
                    po = (pool or psP).tile([128, TT], F32, tag=ptag,
                                            name=f"po{tch}_{ht}")
                    for oc in range(2):
                        nc.tensor.matmul(
                            po[:],
                            anorm[oc][:, tcs],
                            wo_sb[:, oc, hts],
                            start=(oc == 0),
                            stop=(oc == 1),
                        )
                    ob = op_.tile([128, TT], F32, tag="ob")
                    if ob_on_act:
                        nc.scalar.activation(ob[:], po[:], AF.Copy)
                    else:
                        nc.vector.tensor_copy(ob[:], po[:])
                    eng = {"sp": nc.sync, "act": nc.scalar,
                           "pool": nc.gpsimd}[dma_q]
                    eng.dma_start(out[tcs, hts], ob[:])

                def add_outproj_filler(qt):
                    for tch in range(4 * qt, 4 * qt + 4):
                        for ht in range(2):
                            filler.append(
                                lambda tch=tch, ht=ht: outproj_one(tch, ht))

                # deferred per-unit finalize -------------------------------
                # normTS: DVE-only (recips + tensor_scalar into anq tiles);
                # frees the psA slot.  transpose+copy drain via fillers.
                pending = []   # at most one (norm_ts, transp_one, pair, qt)
                last_pacc = [None]

                def make_finalize(pair, qt, pacc):
                    uid = 2 * qt + pair
                    anqs = []

                    def norm_ts():
                        for half in range(2):
                            nc.vector.reciprocal_approx_fast(
                                rec_sb[:, uid, 4 * half : 4 * half + 4],
                                pacc[:, 512 * half + 64 : 512 * half + 324 : 65],
                            )
                        for qc in range(4):
                            anq = np_.tile([128, 128], BF, tag="anq")
                            anqs.append(anq)
                            for hl in range(2):
                                g = hl * 4 + qc
                                nc.vector.tensor_scalar_mul(
                                    anq[:, 64 * hl : 64 * hl + 64],
                                    pacc[:, GOFF[g] : GOFF[g] + 64],
                                    rec_sb[:, uid, g : g + 1],
                                )

                    def transp_one(qc):
                        ptr = psP.tile([128, 128], BF, tag="proj",
                                       name=f"at{pair}_{qt}_{qc}")
                        nc.tensor.transpose(ptr[:], anqs[qc][:], ident[:])
                        nc.vector.tensor_copy(
                            anorm[pair][:, qt * TT + 128 * qc :
                                        qt * TT + 128 * qc + 128],
                            ptr[:],
                        )

                    return norm_ts, transp_one

                def run_pending():
                    if not pending:
                        return
                    norm_ts, transp_one, pair, qt = pending.pop()
                    norm_ts()
                    for qc in range(4):
                        filler.append(lambda qc=qc, t=transp_one: t(qc))
                    if pair == 1:
                        add_outproj_filler(qt)

                def sc_exp(pair, qt, c):
                    cs = bass.ts(c, 128)
                    qts = bass.ts(qt, TT)
                    sc2 = psS.tile([128, 1024], F32, tag="sc",
                                   name=f"sc{pair}_{qt}_{c}")
                    nc.tensor.matmul(
                        sc2[:, 0:TT], k2[0:64, cs], qrot[pair][0:64, qts],
                        start=True, stop=True,
                    )
                    nc.tensor.matmul(
                        sc2[:, TT:1024], k2[64:128, cs],
                        qrot[pair][64:128, qts],
                        start=True, stop=True,
                    )
                    ex = ep.tile([128, 1024], BF, tag="exp")
                    nc.scalar.activation(ex[:], sc2[:], AF.Exp)
                    return ex

                def pv(pacc, ex, c):
                    for g in range(8):
                        hl, qc = divmod(g, 4)
                        # one accumulation group per 2KB PSUM zero region
                        # (bank): only the first group in a bank starts it,
                        # only the last stops it; the other groups' first
                        # writes land on pending-zero bytes and overwrite.
                        nc.tensor.matmul(
                            pacc[:, GOFF[g] : GOFF[g] + 65],
                            ex[:, hl * TT + qc * 128 :
                               hl * TT + qc * 128 + 128],
                            vaug[:, c, :],
                            start=(c == 0 and g in (0, 4)),
                            stop=(c == KC - 1 and g in (3, 7)),
                        )

                def attention_unit(pair, qt, fill_per_kc=2, pre_block=None):
                    """Scores+exp+PV for 2 heads (pair) x 512 queries (qt).

                    Scores are emitted two k-chunks ahead of the PV/filler
                    work so the Activation engine always has a ~2-exp runway
                    against transient PE stalls (psS double-buffering paces
                    the PE to the exp stream automatically)."""
                    if pre_block is not None:
                        pre_block(0)
                    exs = [sc_exp(pair, qt, 0), sc_exp(pair, qt, 1)]
                    run_pending()
                    pacc = psA.tile([128, 1024], F32, tag="att",
                                    name=f"att{pair}_{qt}")
                    for c in range(KC):
                        if pre_block is not None and c % 4 == 0 and c > 0:
                            pre_block(c // 4)
                        if c + 2 < KC:
                            exs.append(sc_exp(pair, qt, c + 2))
                        pv(pacc, exs[c], c)
                        emit_filler(fill_per_kc)
                    pending.append(make_finalize(pair, qt, pacc) + (pair, qt))
                    last_pacc[0] = pacc

                # ---------- emission schedule -----------------------------
                # startup: k/v for tt0, q-pair0 for tt0
                proj_chunk(2, 0)
                qproj_rot(0, 0)
                # tt0 V chunks inline: they only need h(tt0) and fill the PE
                # while RoPE runs; must be emitted before PV(kc0)
                for c in range(4):
                    vproj_one(c)
                # k(tt1) inline too: the PE is otherwise idle until the tt0
                # q-RoPE lands, and its own RoPE then rides right behind on
                # the DVE queue, a full block ahead of the kc4 deadline
                proj_chunk(2, 1)

                # unit (0,0): weave remaining k/v chunks a block ahead of
                # the kc block that needs them
                def pre_block_00(b):
                    if b == 0:
                        add_proj_filler(2, 2)
                        add_vproj_filler(1)
                    elif b == 1:
                        add_proj_filler(2, 3)
                        add_vproj_filler(2)
                    elif b == 2:
                        add_vproj_filler(3)
                        add_qproj_rot_filler(1, 0)

                attention_unit(0, 0, fill_per_kc=3, pre_block=pre_block_00)

                # remaining units; queue proj fillers so qrot[p][tt] is
                # ready one unit ahead
                add_proj_filler(0, 1)
                add_proj_filler(1, 1)
                attention_unit(1, 0)
                add_proj_filler(0, 2)
                attention_unit(0, 1, fill_per_kc=1)
                add_proj_filler(1, 2)
                attention_unit(1, 1, fill_per_kc=1)
                add_proj_filler(0, 3)
                attention_unit(0, 2, fill_per_kc=1)
                add_proj_filler(1, 3)
                attention_unit(1, 2, fill_per_kc=1)
                attention_unit(0, 3, fill_per_kc=1)
                po_pre = {}

                def pre_tail(b):
                    if b == 3:
                        for ht in range(2):
                            po = psP.tile([128, TT], F32, tag="proj",
                                          name=f"po12_{ht}")
                            nc.tensor.matmul(
                                po[:], anorm[0][:, bass.ts(12, 128)],
                                wo_sb[:, 0, bass.ts(ht, TT)],
                                start=True, stop=False,
                            )
                            po_pre[ht] = po

                attention_unit(1, 3, fill_per_kc=2, pre_block=pre_tail)

                # ---------- tail: finalize last unit, outproj(3) ----------
                _, _, lpair, lqt = pending.pop()
                luid = 2 * lqt + lpair
                lpacc = last_pacc[0]
                emit_filler(len(filler))
                for half in range(2):
                    nc.vector.reciprocal_approx_fast(
                        rec_sb[:, luid, 4 * half : 4 * half + 4],
                        lpacc[:, 512 * half + 64 : 512 * half + 324 : 65],
                    )
                for qc in range(4):
                    anq = np_.tile([128, 128], BF, tag="anq")
                    for hl in range(2):
                        g = hl * 4 + qc
                        nc.vector.tensor_scalar_mul(
                            anq[:, 64 * hl : 64 * hl + 64],
                            lpacc[:, GOFF[g] : GOFF[g] + 64],
                            rec_sb[:, luid, g : g + 1],
                        )
                    ptr = psS.tile([128, 128], BF, tag="sc",
                                   name=f"at_t_{qc}")
                    nc.tensor.transpose(ptr[:], anq[:], ident[:])
                    nc.vector.tensor_copy(
                        anorm[lpair][:, lqt * TT + 128 * qc :
                                     lqt * TT + 128 * qc + 128],
                        ptr[:],
                    )
                    tch = 4 * lqt + qc
                    if qc == 0:
                        # tch12 pair-0 half was accumulated during (1,3)
                        for ht in range(2):
                            po = po_pre[ht]
                            hts = bass.ts(ht, TT)
                            nc.tensor.matmul(
                                po[:], anorm[1][:, bass.ts(12, 128)],
                                wo_sb[:, 1, hts],
                                start=False, stop=True,
                            )
                            ob = op_.tile([128, TT], F32, tag="ob")
                            nc.scalar.activation(ob[:], po[:], AF.Copy)
                            eng = nc.sync if ht == 0 else nc.gpsimd
                            eng.dma_start(out[bass.ts(12, 128), hts], ob[:])
                    else:
                        outproj_one(tch, 0, ob_on_act=True, dma_q="sp",
                                    pool=psS, ptag="sc")
                        outproj_one(tch, 1, ob_on_act=False,
                                    dma_q=("pool" if qc < 3 else "act"),
                                    pool=psP)
    nc.finalize()
    return nc


def _get_nc():
    global _nc_cache
    if _nc_cache is None:
        _nc_cache = _build_bass()
    return _nc_cache


def _shard_inputs(hidden_states, cos, sin, w_qkv, w_o):
    """Build per-core input maps. Core c = (b = c // 4, g = c % 4)."""
    cosT = np.ascontiguousarray(cos.T.astype(np.float32))         # [64, S]
    sinT = sin.T.astype(np.float32)
    sinmod = np.concatenate([-sinT[0:32], sinT[32:64]], axis=0)    # sign folded
    sinmod = np.ascontiguousarray(sinmod).astype(_BF16)
    cos2 = np.ascontiguousarray(np.concatenate([cosT, cosT], 0)).astype(_BF16)
    sin2 = np.ascontiguousarray(np.concatenate([sinT, sinT], 0)).astype(_BF16)

    hT = [
        np.ascontiguousarray(hidden_states[b].T).astype(_BF16) for b in range(B)
    ]
    in_maps = []
    for c in range(NCORES):
        b, g = divmod(c, 4)
        q_rows = w_qkv[256 * g : 256 * g + 256] * SCALE
        # rotate-half permuted+negated q rows: row i<32 of each 64-block
        # becomes -row(i+32), row i>=32 becomes +row(i-32)
        qr = q_rows.reshape(4, 2, 32, HID)
        q_rot = np.concatenate([-qr[:, 1], qr[:, 0]], axis=1)      # [4,2,32,H]
        q_rot = q_rot.reshape(256, HID)
        k_rows = w_qkv[1024 + 64 * g : 1024 + 64 * g + 64]
        k_rot = np.concatenate([-k_rows[32:64], k_rows[0:32]], axis=0)
        v_rows = w_qkv[1280 + 64 * g : 1280 + 64 * g + 64]
        wqk = np.concatenate([q_rows, k_rows, v_rows], axis=0)     # [384, 1024]
        wqkT = np.ascontiguousarray(wqk.T).astype(_BF16)           # [1024, 384]
        woT = np.ascontiguousarray(
            w_o[:, 256 * g : 256 * g + 256].T
        ).astype(_BF16)                                            # [256, 1024]
        wqr = np.concatenate([q_rot, k_rot], axis=0)               # [320, 1024]
        wqrT = np.ascontiguousarray(wqr.T).astype(_BF16)           # [1024, 320]
        in_maps.append(
            {
                "hT": hT[b],
                "wqkT": wqkT,
                "wqrd": wqrT,
                "woT": woT,
                "cos2d": cos2,
                "sin2d": sin2,
                "sind": sinmod,
            }
        )
    return in_maps


def _run(inputs, **spmd_kwargs):
    from concourse.bass_utils import run_bass_kernel_spmd

    nc = _get_nc()
    in_maps = _shard_inputs(**inputs)
    res = run_bass_kernel_spmd(
        nc, in_maps, core_ids=list(range(NCORES)), **spmd_kwargs
    )
    outs = []
    for b in range(B):
        acc = res.results[4 * b]["out"].astype(np.float32).copy()
        for g in range(1, 4):
            acc += res.results[4 * b + g]["out"]
        outs.append(acc)
    return np.stack(outs, axis=0), res


def kernel(**inputs):
    out, _ = _run(inputs)
    return out


# revision 7
# speedup vs baseline: 1.0078x; 1.0078x over previous
"""GQA attention layer (QKV proj + RoPE + softmax attention + out proj) on 8
Trainium2 NeuronCores.

Sharding: core c = (batch b, head-group g) with b = c // 4, g = c % 4.
Each core handles one batch and one GQA group (4 q heads + 1 kv head),
computes a w_o-column-sliced partial output (row-parallel), and the host sums
the 4 partials per batch.

v2 redesign (cost-model driven):
- PV matmul flipped to [q-chunk, 65]-shaped output tiles accumulated over k
  chunks: the PE charge per accumulation step drops from N=512 to N=65, and
  softmax normalization becomes a cheap per-partition tensor_scalar instead
  of fp32 broadcast matmuls (4 cyc/row).
- Denominator rides along as the 65th 'ones' column of the V-augmented tile;
  only the first/last group per 2KB PSUM zero-region carries start/stop so
  four accumulation groups can share a bank.
- Normalized [q,d] tiles are transposed back to [d,q] on the PE for the
  output projection.
- The exp stream on the Activation engine (~134us busy) is the critical
  path: everything else hides behind it.  The per-unit softmax finalize is
  split into a DVE-only normalize (emitted after the next unit's first two
  score/exp pairs, so psA turns around without stalling Act) and
  transpose/copy work that drains through the filler queue.
- RoPE: q uses a rotate-projection (the rotate-half permutation folded into
  a second, host-permuted weight matrix) so the rotation is three full-width
  DVE ops off a PSUM read; k uses the classic 4-op form reading PSUM
  directly (PSUM-input elementwise ops may mix partition bases; SBUF/SBUF
  may not, per the BIR verifier).
- V is projected in flipped [token, d] orientation straight into the
  V-augmented tile, eliminating the PE transpose round trip.
- Startup streams inputs over three DMA queues (SP / Activation / Pool
  SWDGE); the tail interleaves the last unit's normalize per q-chunk with
  its output projection across four engines.
"""

import numpy as np
import ml_dtypes

B, S, HID = 2, 2048, 1024
NH, NKV, D = 16, 4, 64
SCALE = D ** -0.5
NCORES = 8
TT = 512          # token tile (projection N / q tile)
NTT = S // TT     # 4
KC = S // 128     # 16 k chunks

_BF16 = ml_dtypes.bfloat16

_nc_cache = None


def _build_bass():
    import concourse.bass as bass
    import concourse.mybir as mybir
    import concourse.tile as tile
    from concourse import bacc
    from concourse.masks import make_identity

    BF = mybir.dt.bfloat16
    F32 = mybir.dt.float32
    AF = mybir.ActivationFunctionType
    MULT = mybir.AluOpType.mult
    ADD = mybir.AluOpType.add

    nc = bacc.Bacc()
    hT = nc.dram_tensor("hT", (HID, S), BF, kind="ExternalInput")
    wqkT = nc.dram_tensor("wqkT", (HID, 384), BF, kind="ExternalInput")
    woT = nc.dram_tensor("woT", (256, HID), BF, kind="ExternalInput")
    cos2d = nc.dram_tensor("cos2d", (128, S), BF, kind="ExternalInput")
    sin2d = nc.dram_tensor("sin2d", (128, S), BF, kind="ExternalInput")
    sind = nc.dram_tensor("sind", (64, S), BF, kind="ExternalInput")
    wqrd = nc.dram_tensor("wqrd", (HID, 320), BF, kind="ExternalInput")
    out = nc.dram_tensor("out", (S, HID), F32, kind="ExternalOutput")

    # psA group layout: 8 groups of [64 attn | 1 den] columns; keep each
    # group's 65 columns inside one 2KB PSUM bank (g<4 bank0, g>=4 bank1)
    GOFF = [65 * g if g < 4 else 512 + 65 * (g - 4) for g in range(8)]

    with tile.TileContext(nc) as tc:
        with (
            tc.tile_pool(name="persist", bufs=1) as pp,
            tc.tile_pool(name="rope", bufs=3) as rp,
            tc.tile_pool(name="exps", bufs=8) as ep,
            tc.tile_pool(name="norm", bufs=8) as np_,
            tc.tile_pool(name="outsb", bufs=4) as op_,
        ):
            # ---- persistent SBUF tiles + input loads (chunked for DMA spread)
            h_sb = pp.tile([128, 8, S], BF, tag="h_sb")
            wqk_sb = pp.tile([128, 8, 384], BF, tag="wqk_sb")
            wqr_sb = pp.tile([128, 8, 320], BF, tag="wqr_sb")
            wo_sb = pp.tile([128, 2, HID], BF, tag="wo_sb")
            # cos/sin duplicated across both partition halves (full-width
            # rot-projection RoPE); sin_sb is the sign-folded classic table
            cos2_sb = pp.tile([128, S], BF, tag="cos2_sb")
            sin2_sb = pp.tile([128, S], BF, tag="sin2_sb")
            cos_sb = cos2_sb[0:64, :]
            sin_sb = pp.tile([64, S], BF, tag="sin_sb")
            h_dram = hT.rearrange("(c p) s -> p c s", p=128)
            wqk_dram = wqkT.rearrange("(c p) r -> p c r", p=128)
            wqr_dram = wqrd.rearrange("(c p) r -> p c r", p=128)
            wo_dram = woT.rearrange("(c p) h -> p c h", p=128)
            # three-queue head: h chunks stream on SP (500ns/issue), wqk
            # splits across the Activation HWDGE queue (after the first
            # cos/sin quarter that RoPE tt0 needs) and the Pool SWDGE queue,
            # so the first rc2 projection is fed from ~1.5us on
            nc.sync.dma_start(wqk_sb[:, 0, :], wqk_dram[:, 0, :])
            nc.sync.dma_start(wqk_sb[:, 1, :], wqk_dram[:, 1, :])
            for hc in range(2, 4):
                nc.scalar.dma_start(wqk_sb[:, hc, :], wqk_dram[:, hc, :])
            nc.scalar.dma_start(cos2_sb[:, 0:TT], cos2d[:, 0:TT])
            nc.scalar.dma_start(sin2_sb[:, 0:TT], sin2d[:, 0:TT])
            nc.scalar.dma_start(sin_sb[:, 0:TT], sind[:, 0:TT])
            nc.scalar.dma_start(wqr_sb[:, :, 256:320],
                                wqr_dram[:, :, 256:320])
            nc.scalar.dma_start(wqr_sb[:, :, 0:128],
                                wqr_dram[:, :, 0:128])
            for hc in range(4, 8):
                nc.gpsimd.dma_start(wqk_sb[:, hc, :], wqk_dram[:, hc, :])
            for hc in range(8):
                nc.sync.dma_start(h_sb[:, hc, 0:TT], h_dram[:, hc, 0:TT])
            for hc in range(8):
                nc.sync.dma_start(h_sb[:, hc, TT : 2 * TT],
                                  h_dram[:, hc, TT : 2 * TT])
            # h(tt2) rides the Activation HWDGE queue, which is idle after
            # the table loads until the first exp (~13us)
            for hc in range(8):
                nc.scalar.dma_start(h_sb[:, hc, 2 * TT : 3 * TT],
                                    h_dram[:, hc, 2 * TT : 3 * TT])
            for hc in range(8):
                nc.sync.dma_start(h_sb[:, hc, 3 * TT : S],
                                  h_dram[:, hc, 3 * TT : S])
            # second halves of the RoPE tables ride the idle Pool SWDGE
            # queue (land ~10us, needed from ~13us on)
            nc.gpsimd.dma_start(cos2_sb[:, TT:S], cos2d[:, TT:S])
            nc.gpsimd.dma_start(sin2_sb[:, TT:S], sin2d[:, TT:S])
            nc.gpsimd.dma_start(sin_sb[:, TT:S], sind[:, TT:S])
            nc.gpsimd.dma_start(wqr_sb[:, :, 128:256],
                                wqr_dram[:, :, 128:256])
            for oc in range(2):
                nc.sync.dma_start(wo_sb[:, oc, :], wo_dram[:, oc, :])

            ident = pp.tile([128, 128], BF, tag="ident")
            make_identity(nc, ident[:])
            # preload the exp table set while input DMAs stream
            warm = pp.tile([1, 8], F32, tag="warm")
            nc.any.memset(warm[:], 0.0)
            nc.scalar.activation(warm[:], warm[:], AF.Exp)

            # roped q, 2 heads per tile (head 2p at rows 0:64, 2p+1 at 64:128)
            qrot = [pp.tile([128, S], BF, tag=f"qrot{p}", name=f"qrot{p}") for p in range(2)]
            # roped k duplicated on both partition halves (matmul requires
            # lhsT/rhs base partitions to match; BIR forbids cross-base
            # SBUF/SBUF elementwise reads, so the halves are materialized)
            k2 = pp.tile([128, S], BF, tag="k2")
            # V with ones column for the fused softmax denominator
            vaug = pp.tile([128, KC, 65], BF, tag="vaug")
            nc.any.memset(vaug[:], 1.0)
            # normalized attention output in [d, token] layout per pair
            anorm = [pp.tile([128, S], BF, tag=f"anorm{o}", name=f"anorm{o}") for o in range(2)]
            # per-unit reciprocal denominators (unit = pair + 2*qt)
            rec_sb = pp.tile([128, 8, 8], F32, tag="rec_sb")

            def rope64(ps_blk, dest, tts, tmp_tag):
                """RoPE one 64-row head block [64, TT] read directly from
                PSUM (PSUM-input elementwise ops may mix partition bases;
                SBUF/SBUF may not). The bf16 t1+rt add runs in DVE 2x."""
                t1 = rp.tile([64, TT], BF, tag=f"t1{tmp_tag}")
                rt = rp.tile([64, TT], BF, tag=f"rt{tmp_tag}")
                nc.vector.tensor_tensor(t1[:], ps_blk[0:64, :], cos_sb[:, tts], MULT)
                nc.vector.tensor_tensor(
                    rt[0:32, :], ps_blk[32:64, :], sin_sb[0:32, tts], MULT
                )
                nc.vector.tensor_tensor(
                    rt[32:64, :], ps_blk[0:32, :], sin_sb[32:64, tts], MULT
                )
                nc.vector.tensor_tensor(dest, t1[:], rt[:], ADD)

            with (
                tc.tile_pool(name="psP", bufs=2, space="PSUM") as psP,
                tc.tile_pool(name="psS", bufs=2, space="PSUM") as psS,
                tc.tile_pool(name="psACC", bufs=1, space="PSUM") as psA,
            ):
                def proj_finish(rc, tt, ps, on_act=False):
                    tts = bass.ts(tt, TT)
                    if rc == 2:
                        rope64(ps[0:64, :], k2[0:64, tts], tts, "k")
                        nc.gpsimd.tensor_copy(k2[64:128, tts], k2[0:64, tts])
                    else:
                        rope64(ps[0:64, :], qrot[rc][0:64, tts], tts, "qa")
                        rope64(ps[64:128, :], qrot[rc][64:128, tts], tts, "qb")

                def wqk_cols(rc):
                    # q pairs at 0:128 / 128:256; k-only at 256:320
                    return slice(256, 320) if rc == 2 else bass.ts(rc, 128)

                def proj_chunk(rc, tt):
                    """One projection tile: 8 accumulating matmuls, then RoPE
                    (rc<2: q pair [128,TT]; rc==2: k rows only [64,TT])."""
                    tts = bass.ts(tt, TT)
                    rows = 64 if rc == 2 else 128
                    ps = psP.tile([rows, TT], F32, tag="proj",
                                  name=f"proj{rc}_{tt}")
                    for hc in range(8):
                        nc.tensor.matmul(
                            ps[:],
                            wqk_sb[:, hc, wqk_cols(rc)],
                            h_sb[:, hc, tts],
                            start=(hc == 0),
                            stop=(hc == 7),
                        )
                    proj_finish(rc, tt, ps)

                def kproj_rot(tt):
                    """k projection + rotate-half projection; k2 rows 0:64
                    via three full-width DVE ops, then the partition-half
                    duplicate for the head-B score lhsT."""
                    tts = bass.ts(tt, TT)
                    ps = psP.tile([64, TT], F32, tag="proj", name=f"kp{tt}")
                    ps2 = psP.tile([64, TT], F32, tag="proj", name=f"kr{tt}")
                    for hc in range(8):
                        nc.tensor.matmul(
                            ps[:], wqk_sb[:, hc, 256:320], h_sb[:, hc, tts],
                            start=(hc == 0), stop=(hc == 7),
                        )
                    for hc in range(8):
                        nc.tensor.matmul(
                            ps2[:], wqr_sb[:, hc, 256:320], h_sb[:, hc, tts],
                            start=(hc == 0), stop=(hc == 7),
                        )
                    t1 = rp.tile([64, TT], BF, tag="kt1")
                    t2 = rp.tile([64, TT], BF, tag="kt2")
                    nc.vector.tensor_tensor(t1[:], ps[:], cos2_sb[0:64, tts], MULT)
                    nc.vector.tensor_tensor(t2[:], ps2[:], sin2_sb[0:64, tts], MULT)
                    nc.vector.tensor_tensor(k2[0:64, tts], t1[:], t2[:], ADD)
                    nc.gpsimd.tensor_copy(k2[64:128, tts], k2[0:64, tts])

                def add_kproj_rot_filler(tt):
                    tts = bass.ts(tt, TT)
                    cell = {}

                    def mk(which, h0):
                        def go():
                            if which == 0 and h0 == 0:
                                cell["ps"] = psP.tile(
                                    [64, TT], F32, tag="proj", name=f"kp{tt}")
                                cell["ps2"] = psP.tile(
                                    [64, TT], F32, tag="proj", name=f"kr{tt}")
                            w = wqk_sb if which == 0 else wqr_sb
                            ps = cell["ps"] if which == 0 else cell["ps2"]
                            for hc in (h0, h0 + 1):
                                nc.tensor.matmul(
                                    ps[:], w[:, hc, 256:320],
                                    h_sb[:, hc, tts],
                                    start=(hc == 0), stop=(hc == 7),
                                )
                            if which == 1 and h0 == 6:
                                t1 = rp.tile([64, TT], BF, tag="kt1")
                                t2 = rp.tile([64, TT], BF, tag="kt2")
                                nc.vector.tensor_tensor(
                                    t1[:], cell["ps"][:],
                                    cos2_sb[0:64, tts], MULT)
                                nc.vector.tensor_tensor(
                                    t2[:], cell["ps2"][:],
                                    sin2_sb[0:64, tts], MULT)
                                nc.vector.tensor_tensor(
                                    k2[0:64, tts], t1[:], t2[:], ADD)
                                nc.gpsimd.tensor_copy(
                                    k2[64:128, tts], k2[0:64, tts])
                        return go

                    for which in (0, 1):
                        for h0 in (0, 2, 4, 6):
                            filler.append(mk(which, h0))

                def qproj_rot(rc, tt):
                    """q projection with the rotate-half permutation folded
                    into a second (host-permuted) weight matrix: qrot =
                    q*cos + rot(q)*sin in three full-width DVE ops."""
                    tts = bass.ts(tt, TT)
                    ps = psP.tile([128, TT], F32, tag="proj",
                                  name=f"qp{rc}_{tt}")
                    ps2 = psP.tile([128, TT], F32, tag="proj",
                                   name=f"qr{rc}_{tt}")
                    for hc in range(8):
                        nc.tensor.matmul(
                            ps[:], wqk_sb[:, hc, bass.ts(rc, 128)],
                            h_sb[:, hc, tts],
                            start=(hc == 0), stop=(hc == 7),
                        )
                    for hc in range(8):
                        nc.tensor.matmul(
                            ps2[:], wqr_sb[:, hc, bass.ts(rc, 128)],
                            h_sb[:, hc, tts],
                            start=(hc == 0), stop=(hc == 7),
                        )
                    t1 = rp.tile([128, TT], BF, tag="rt1")
                    t2 = rp.tile([128, TT], BF, tag="rt2")
                    nc.vector.tensor_tensor(t1[:], ps[:], cos2_sb[:, tts], MULT)
                    nc.vector.tensor_tensor(t2[:], ps2[:], sin2_sb[:, tts], MULT)
                    nc.vector.tensor_tensor(qrot[rc][:, tts], t1[:], t2[:], ADD)

                def add_qproj_rot_filler(rc, tt):
                    tts = bass.ts(tt, TT)
                    cell = {}

                    def mk(which, h0):
                        def go():
                            if which == 0 and h0 == 0:
                                cell["ps"] = psP.tile(
                                    [128, TT], F32, tag="proj",
                                    name=f"qp{rc}_{tt}")
                                cell["ps2"] = psP.tile(
                                    [128, TT], F32, tag="proj",
                                    name=f"qr{rc}_{tt}")
                            w = wqk_sb if which == 0 else wqr_sb
                            ps = cell["ps"] if which == 0 else cell["ps2"]
                            for hc in (h0, h0 + 1):
                                nc.tensor.matmul(
                                    ps[:], w[:, hc, bass.ts(rc, 128)],
                                    h_sb[:, hc, tts],
                                    start=(hc == 0), stop=(hc == 7),
                                )
                            if which == 1 and h0 == 6:
                                t1 = rp.tile([128, TT], BF, tag="rt1")
                                t2 = rp.tile([128, TT], BF, tag="rt2")
                                nc.vector.tensor_tensor(
                                    t1[:], cell["ps"][:], cos2_sb[:, tts], MULT)
                                nc.vector.tensor_tensor(
                                    t2[:], cell["ps2"][:], sin2_sb[:, tts], MULT)
                                nc.vector.tensor_tensor(
                                    qrot[rc][:, tts], t1[:], t2[:], ADD)
                        return go

                    for which in (0, 1):
                        for h0 in (0, 2, 4, 6):
                            filler.append(mk(which, h0))

                def vproj_one(c):
                    # V projection flipped to [token, d] orientation: writes
                    # the vaug [kpos, d] layout directly (no PE transpose)
                    tcs = bass.ts(c, 128)
                    vp = psP.tile([128, 64], F32, tag="proj", name=f"vp{c}")
                    for hc in range(8):
                        nc.tensor.matmul(
                            vp[:],
                            h_sb[:, hc, tcs],
                            wqk_sb[:, hc, 320:384],
                            start=(hc == 0),
                            stop=(hc == 7),
                        )
                    nc.vector.tensor_copy(vaug[:, c, 0:64], vp[:])

                # filler queue: zero-arg closures, each emitting ~1-2 PE
                # matmuls (plus attached vector/pool work)
                filler = []

                def emit_filler(n):
                    for _ in range(n):
                        if not filler:
                            return
                        filler.pop(0)()

                def add_proj_filler(rc, tt):
                    # split one proj chunk into 4 closures of 2 matmuls each
                    tts = bass.ts(tt, TT)
                    cell = {}

                    def mk(h0):
                        def go():
                            if h0 == 0:
                                cell["ps"] = psP.tile(
                                    [64 if rc == 2 else 128, TT], F32,
                                    tag="proj", name=f"proj{rc}_{tt}",
                                )
                            ps = cell["ps"]
                            for hc in (h0, h0 + 1):
                                nc.tensor.matmul(
                                    ps[:],
                                    wqk_sb[:, hc, wqk_cols(rc)],
                                    h_sb[:, hc, tts],
                                    start=(hc == 0),
                                    stop=(hc == 7),
                                )
                            if h0 == 6:
                                proj_finish(rc, tt, ps)
                        return go

                    for h0 in (0, 2, 4, 6):
                        filler.append(mk(h0))

                def add_vproj_filler(tt):
                    for c in range(4 * tt, 4 * tt + 4):
                        filler.append(lambda c=c: vproj_one(c))

                def outproj_one(tch, ht, ob_on_act=False, dma_q="sp",
                                pool=None, ptag="proj"):
                    tcs = bass.ts(tch, 128)
                    hts = bass.ts(ht, TT)
                    po = (pool or psP).tile([128, TT], F32, tag=ptag,
                                            name=f"po{tch}_{ht}")
                    for oc in range(2):
                        nc.tensor.matmul(
                            po[:],
                            anorm[oc][:, tcs],
                            wo_sb[:, oc, hts],
                            start=(oc == 0),
                            stop=(oc == 1),
                        )
                    ob = op_.tile([128, TT], F32, tag="ob")
                    if ob_on_act:
                        nc.scalar.activation(ob[:], po[:], AF.Copy)
                    else:
                        nc.vector.tensor_copy(ob[:], po[:])
                    eng = {"sp": nc.sync, "act": nc.scalar,
                           "pool": nc.gpsimd}[dma_q]
                    eng.dma_start(out[tcs, hts], ob[:])

                def add_outproj_filler(qt):
                    for tch in range(4 * qt, 4 * qt + 4):
                        for ht in range(2):
                            filler.append(
                                lambda tch=tch, ht=ht: outproj_one(tch, ht))

                # deferred per-unit finalize -------------------------------
                # normTS: DVE-only (recips + tensor_scalar into anq tiles);
                # frees the psA slot.  transpose+copy drain via fillers.
                pending = []   # at most one (norm_ts, transp_one, pair, qt)
                last_pacc = [None]

                def make_finalize(pair, qt, pacc):
                    uid = 2 * qt + pair
                    anqs = []

                    def norm_ts():
                        for half in range(2):
                            nc.vector.reciprocal_approx_fast(
                                rec_sb[:, uid, 4 * half : 4 * half + 4],
                                pacc[:, 512 * half + 64 : 512 * half + 324 : 65],
                            )
                        for qc in range(4):
                            anq = np_.tile([128, 128], BF, tag="anq")
                            anqs.append(anq)
                            for hl in range(2):
                                g = hl * 4 + qc
                                nc.vector.tensor_scalar_mul(
                                    anq[:, 64 * hl : 64 * hl + 64],
                                    pacc[:, GOFF[g] : GOFF[g] + 64],
                                    rec_sb[:, uid, g : g + 1],
                                )

                    def transp_one(qc):
                        ptr = psP.tile([128, 128], BF, tag="proj",
                                       name=f"at{pair}_{qt}_{qc}")
                        nc.tensor.transpose(ptr[:], anqs[qc][:], ident[:])
                        nc.vector.tensor_copy(
                            anorm[pair][:, qt * TT + 128 * qc :
                                        qt * TT + 128 * qc + 128],
                            ptr[:],
                        )

                    return norm_ts, transp_one

                def run_pending():
                    if not pending:
                        return
                    norm_ts, transp_one, pair, qt = pending.pop()
                    norm_ts()
                    for qc in range(4):
                        filler.append(lambda qc=qc, t=transp_one: t(qc))
                    if pair == 1:
                        add_outproj_filler(qt)

                def sc_exp(pair, qt, c):
                    cs = bass.ts(c, 128)
                    qts = bass.ts(qt, TT)
                    sc2 = psS.tile([128, 1024], F32, tag="sc",
                                   name=f"sc{pair}_{qt}_{c}")
                    nc.tensor.matmul(
                        sc2[:, 0:TT], k2[0:64, cs], qrot[pair][0:64, qts],
                        start=True, stop=True,
                    )
                    nc.tensor.matmul(
                        sc2[:, TT:1024], k2[64:128, cs],
                        qrot[pair][64:128, qts],
                        start=True, stop=True,
                    )
                    ex = ep.tile([128, 1024], BF, tag="exp")
                    nc.scalar.activation(ex[:], sc2[:], AF.Exp)
                    return ex

                def pv(pacc, ex, c):
                    for g in range(8):
                        hl, qc = divmod(g, 4)
                        # one accumulation group per 2KB PSUM zero region
                        # (bank): only the first group in a bank starts it,
                        # only the last stops it; the other groups' first
                        # writes land on pending-zero bytes and overwrite.
                        nc.tensor.matmul(
                            pacc[:, GOFF[g] : GOFF[g] + 65],
                            ex[:, hl * TT + qc * 128 :
                               hl * TT + qc * 128 + 128],
                            vaug[:, c, :],
                            start=(c == 0 and g in (0, 4)),
                            stop=(c == KC - 1 and g in (3, 7)),
                        )

                def attention_unit(pair, qt, fill_per_kc=2, pre_block=None):
                    """Scores+exp+PV for 2 heads (pair) x 512 queries (qt).

                    Scores are emitted two k-chunks ahead of the PV/filler
                    work so the Activation engine always has a ~2-exp runway
                    against transient PE stalls (psS double-buffering paces
                    the PE to the exp stream automatically)."""
                    if pre_block is not None:
                        pre_block(0)
                    exs = [sc_exp(pair, qt, 0), sc_exp(pair, qt, 1)]
                    run_pending()
                    pacc = psA.tile([128, 1024], F32, tag="att",
                                    name=f"att{pair}_{qt}")
                    for c in range(KC):
                        if pre_block is not None and c % 4 == 0 and c > 0:
                            pre_block(c // 4)
                        if c + 2 < KC:
                            exs.append(sc_exp(pair, qt, c + 2))
                        pv(pacc, exs[c], c)
                        emit_filler(fill_per_kc)
                    pending.append(make_finalize(pair, qt, pacc) + (pair, qt))
                    last_pacc[0] = pacc

                # ---------- emission schedule -----------------------------
                # startup: k/v for tt0, q-pair0 for tt0
                proj_chunk(2, 0)
                qproj_rot(0, 0)
                # tt0 V chunks inline: they only need h(tt0) and fill the PE
                # while RoPE runs; must be emitted before PV(kc0)
                for c in range(4):
                    vproj_one(c)
                # k(tt1) inline too: the PE is otherwise idle until the tt0
                # q-RoPE lands, and its own RoPE then rides right behind on
                # the DVE queue, a full block ahead of the kc4 deadline
                proj_chunk(2, 1)

                # unit (0,0): weave remaining k/v chunks a block ahead of
                # the kc block that needs them
                def pre_block_00(b):
                    if b == 0:
                        add_proj_filler(2, 2)
                        add_vproj_filler(1)
                    elif b == 1:
                        add_proj_filler(2, 3)
                        add_vproj_filler(2)
                    elif b == 2:
                        add_vproj_filler(3)
                        add_qproj_rot_filler(1, 0)

                attention_unit(0, 0, fill_per_kc=3, pre_block=pre_block_00)

                # remaining units; queue proj fillers so qrot[p][tt] is
                # ready one unit ahead
                add_proj_filler(0, 1)
                add_proj_filler(1, 1)
                attention_unit(1, 0)
                add_proj_filler(0, 2)
                attention_unit(0, 1, fill_per_kc=1)
                add_proj_filler(1, 2)
                attention_unit(1, 1, fill_per_kc=1)
                add_proj_filler(0, 3)
                attention_unit(0, 2, fill_per_kc=1)
                add_proj_filler(1, 3)
                attention_unit(1, 2, fill_per_kc=1)
                attention_unit(0, 3, fill_per_kc=1)
                po_pre = {}

                def pre_tail(b):
                    if b == 3:
                        for ht in range(2):
                            po = psP.tile([128, TT], F32, tag="proj",
                                          name=f"po12_{ht}")
                            nc.tensor.matmul(
                                po[:], anorm[0][:, bass.ts(12, 128)],
                                wo_sb[:, 0, bass.ts(ht, TT)],
                                start=True, stop=False,
                            )
                            po_pre[ht] = po

                attention_unit(1, 3, fill_per_kc=2, pre_block=pre_tail)

                # ---------- tail: finalize last unit, outproj(3) ----------
                _, _, lpair, lqt = pending.pop()
                luid = 2 * lqt + lpair
                lpacc = last_pacc[0]
                emit_filler(len(filler))
                for half in range(2):
                    nc.vector.reciprocal_approx_fast(
                        rec_sb[:, luid, 4 * half : 4 * half + 4],
                        lpacc[:, 512 * half + 64 : 512 * half + 324 : 65],
                    )
                for qc in range(4):
                    anq = np_.tile([128, 128], BF, tag="anq")
                    for hl in range(2):
                        g = hl * 4 + qc
                        nc.vector.tensor_scalar_mul(
                            anq[:, 64 * hl : 64 * hl + 64],
                            lpacc[:, GOFF[g] : GOFF[g] + 64],
                            rec_sb[:, luid, g : g + 1],
                        )
                    ptr = psS.tile([128, 128], BF, tag="sc",
                                   name=f"at_t_{qc}")
                    nc.tensor.transpose(ptr[:], anq[:], ident[:])
                    nc.vector.tensor_copy(
                        anorm[lpair][:, lqt * TT + 128 * qc :
                                     lqt * TT + 128 * qc + 128],
                        ptr[:],
                    )
                    tch = 4 * lqt + qc
                    if qc == 0:
                        # tch12 pair-0 half was accumulated during (1,3)
                        for ht in range(2):
                            po = po_pre[ht]
                            hts = bass.ts(ht, TT)
                            nc.tensor.matmul(
                                po[:], anorm[1][:, bass.ts(12, 128)],
                                wo_sb[:, 1, hts],
                                start=False, stop=True,
                            )
                            ob = op_.tile([128, TT], F32, tag="ob")
                            nc.scalar.activation(ob[:], po[:], AF.Copy)
                            eng = nc.sync if ht == 0 else nc.gpsimd
                            eng.dma_start(out[bass.ts(12, 128), hts], ob[:])
                    else:
                        outproj_one(tch, 0, ob_on_act=True, dma_q="sp",
                                    pool=psS, ptag="sc")
                        outproj_one(tch, 1, ob_on_act=False,
                                    dma_q=("pool" if qc < 3 else "act"),
                                    pool=psP)
    nc.finalize()
    return nc


def _get_nc():
    global _nc_cache
    if _nc_cache is None:
        _nc_cache = _build_bass()
    return _nc_cache


def _shard_inputs(hidden_states, cos, sin, w_qkv, w_o):
    """Build per-core input maps. Core c = (b = c // 4, g = c % 4)."""
    cosT = np.ascontiguousarray(cos.T.astype(np.float32))         # [64, S]
    sinT = sin.T.astype(np.float32)
    sinmod = np.concatenate([-sinT[0:32], sinT[32:64]], axis=0)    # sign folded
    sinmod = np.ascontiguousarray(sinmod).astype(_BF16)
    cos2 = np.ascontiguousarray(np.concatenate([cosT, cosT], 0)).astype(_BF16)
    sin2 = np.ascontiguousarray(np.concatenate([sinT, sinT], 0)).astype(_BF16)

    hT = [
        np.ascontiguousarray(hidden_states[b].T).astype(_BF16) for b in range(B)
    ]
    in_maps = []
    for c in range(NCORES):
        b, g = divmod(c, 4)
        q_rows = w_qkv[256 * g : 256 * g + 256] * SCALE
        # rotate-half permuted+negated q rows: row i<32 of each 64-block
        # becomes -row(i+32), row i>=32 becomes +row(i-32)
        qr = q_rows.reshape(4, 2, 32, HID)
        q_rot = np.concatenate([-qr[:, 1], qr[:, 0]], axis=1)      # [4,2,32,H]
        q_rot = q_rot.reshape(256, HID)
        k_rows = w_qkv[1024 + 64 * g : 1024 + 64 * g + 64]
        k_rot = np.concatenate([-k_rows[32:64], k_rows[0:32]], axis=0)
        v_rows = w_qkv[1280 + 64 * g : 1280 + 64 * g + 64]
        wqk = np.concatenate([q_rows, k_rows, v_rows], axis=0)     # [384, 1024]
        wqkT = np.ascontiguousarray(wqk.T).astype(_BF16)           # [1024, 384]
        woT = np.ascontiguousarray(
            w_o[:, 256 * g : 256 * g + 256].T
        ).astype(_BF16)                                            # [256, 1024]
        wqr = np.concatenate([q_rot, k_rot], axis=0)               # [320, 1024]
        wqrT = np.ascontiguousarray(wqr.T).astype(_BF16)           # [1024, 320]
        in_maps.append(
            {
                "hT": hT[b],
                "wqkT": wqkT,
                "wqrd": wqrT,
                "woT": woT,
                "cos2d": cos2,
                "sin2d": sin2,
                "sind": sinmod,
            }
        )
    return in_maps


def _run(inputs, **spmd_kwargs):
    from concourse.bass_utils import run_bass_kernel_spmd

    nc = _get_nc()
    in_maps = _shard_inputs(**inputs)
    res = run_bass_kernel_spmd(
        nc, in_maps, core_ids=list(range(NCORES)), **spmd_kwargs
    )
    outs = []
    for b in range(B):
        acc = res.results[4 * b]["out"].astype(np.float32).copy()
        for g in range(1, 4):
            acc += res.results[4 * b + g]["out"]
        outs.append(acc)
    return np.stack(outs, axis=0), res


def kernel(**inputs):
    out, _ = _run(inputs)
    return out


# revision 9
# speedup vs baseline: 1.0169x; 1.0091x over previous
"""GQA attention layer (QKV proj + RoPE + softmax attention + out proj) on 8
Trainium2 NeuronCores.

Sharding: core c = (batch b, head-group g) with b = c // 4, g = c % 4.
Each core handles one batch and one GQA group (4 q heads + 1 kv head),
computes a w_o-column-sliced partial output (row-parallel), and the host sums
the 4 partials per batch.

v2 redesign (cost-model driven):
- PV matmul flipped to [q-chunk, 65]-shaped output tiles accumulated over k
  chunks: the PE charge per accumulation step drops from N=512 to N=65, and
  softmax normalization becomes a cheap per-partition tensor_scalar instead
  of fp32 broadcast matmuls (4 cyc/row).
- Denominator rides along as the 65th 'ones' column of the V-augmented tile;
  only the first/last group per 2KB PSUM zero-region carries start/stop so
  four accumulation groups can share a bank.
- Normalized [q,d] tiles are transposed back to [d,q] on the PE for the
  output projection.
- The exp stream on the Activation engine (~134us busy) is the critical
  path: everything else hides behind it.  The per-unit softmax finalize is
  split into a DVE-only normalize (emitted after the next unit's first two
  score/exp pairs, so psA turns around without stalling Act) and
  transpose/copy work that drains through the filler queue.
- RoPE: q uses a rotate-projection (the rotate-half permutation folded into
  a second, host-permuted weight matrix) so the rotation is three full-width
  DVE ops off a PSUM read; k uses the classic 4-op form reading PSUM
  directly (PSUM-input elementwise ops may mix partition bases; SBUF/SBUF
  may not, per the BIR verifier).
- V is projected in flipped [token, d] orientation straight into the
  V-augmented tile, eliminating the PE transpose round trip.
- Startup streams inputs over three DMA queues (SP / Activation / Pool
  SWDGE); the tail interleaves the last unit's normalize per q-chunk with
  its output projection across four engines.
"""

import numpy as np
import ml_dtypes

B, S, HID = 2, 2048, 1024
NH, NKV, D = 16, 4, 64
SCALE = D ** -0.5
NCORES = 8
TT = 512          # token tile (projection N / q tile)
NTT = S // TT     # 4
KC = S // 128     # 16 k chunks

_BF16 = ml_dtypes.bfloat16

_nc_cache = None


def _build_bass():
    import concourse.bass as bass
    import concourse.mybir as mybir
    import concourse.tile as tile
    from concourse import bacc
    from concourse.masks import make_identity

    BF = mybir.dt.bfloat16
    F32 = mybir.dt.float32
    AF = mybir.ActivationFunctionType
    MULT = mybir.AluOpType.mult
    ADD = mybir.AluOpType.add

    nc = bacc.Bacc()
    hT = nc.dram_tensor("hT", (HID, S), BF, kind="ExternalInput")
    wqkT = nc.dram_tensor("wqkT", (HID, 384), BF, kind="ExternalInput")
    woT = nc.dram_tensor("woT", (256, HID), BF, kind="ExternalInput")
    cos2d = nc.dram_tensor("cos2d", (128, S), BF, kind="ExternalInput")
    sin2d = nc.dram_tensor("sin2d", (128, S), BF, kind="ExternalInput")
    sind = nc.dram_tensor("sind", (64, S), BF, kind="ExternalInput")
    wqrd = nc.dram_tensor("wqrd", (HID, 320), BF, kind="ExternalInput")
    out = nc.dram_tensor("out", (S, HID), F32, kind="ExternalOutput")

    # psA group layout: 8 groups of [64 attn | 1 den] columns; keep each
    # group's 65 columns inside one 2KB PSUM bank (g<4 bank0, g>=4 bank1)
    GOFF = [65 * g if g < 4 else 512 + 65 * (g - 4) for g in range(8)]

    with tile.TileContext(nc) as tc:
        with (
            tc.tile_pool(name="persist", bufs=1) as pp,
            tc.tile_pool(name="rope", bufs=3) as rp,
            tc.tile_pool(name="exps", bufs=8) as ep,
            tc.tile_pool(name="norm", bufs=8) as np_,
            tc.tile_pool(name="outsb", bufs=4) as op_,
        ):
            # ---- persistent SBUF tiles + input loads (chunked for DMA spread)
            h_sb = pp.tile([128, 8, S], BF, tag="h_sb")
            wqk_sb = pp.tile([128, 8, 384], BF, tag="wqk_sb")
            wqr_sb = pp.tile([128, 8, 320], BF, tag="wqr_sb")
            wo_sb = pp.tile([128, 2, HID], BF, tag="wo_sb")
            # cos/sin duplicated across both partition halves (full-width
            # rot-projection RoPE); sin_sb is the sign-folded classic table
            cos2_sb = pp.tile([128, S], BF, tag="cos2_sb")
            sin2_sb = pp.tile([128, S], BF, tag="sin2_sb")
            cos_sb = cos2_sb[0:64, :]
            sin_sb = pp.tile([64, S], BF, tag="sin_sb")
            h_dram = hT.rearrange("(c p) s -> p c s", p=128)
            wqk_dram = wqkT.rearrange("(c p) r -> p c r", p=128)
            wqr_dram = wqrd.rearrange("(c p) r -> p c r", p=128)
            wo_dram = woT.rearrange("(c p) h -> p c h", p=128)
            # three-queue head: h chunks stream on SP (500ns/issue), wqk
            # splits across the Activation HWDGE queue (after the first
            # cos/sin quarter that RoPE tt0 needs) and the Pool SWDGE queue,
            # so the first rc2 projection is fed from ~1.5us on
            nc.sync.dma_start(wqk_sb[:, 0, :], wqk_dram[:, 0, :])
            nc.sync.dma_start(wqk_sb[:, 1, :], wqk_dram[:, 1, :])
            for hc in range(2, 4):
                nc.scalar.dma_start(wqk_sb[:, hc, :], wqk_dram[:, hc, :])
            nc.scalar.dma_start(cos2_sb[:, 0:TT], cos2d[:, 0:TT])
            nc.scalar.dma_start(sin2_sb[:, 0:TT], sin2d[:, 0:TT])
            nc.scalar.dma_start(sin_sb[:, 0:TT], sind[:, 0:TT])
            nc.scalar.dma_start(wqr_sb[:, :, 256:320],
                                wqr_dram[:, :, 256:320])
            nc.scalar.dma_start(wqr_sb[:, :, 0:128],
                                wqr_dram[:, :, 0:128])
            for hc in range(4, 8):
                nc.gpsimd.dma_start(wqk_sb[:, hc, :], wqk_dram[:, hc, :])
            for hc in range(8):
                nc.sync.dma_start(h_sb[:, hc, 0:TT], h_dram[:, hc, 0:TT])
            for hc in range(8):
                nc.sync.dma_start(h_sb[:, hc, TT : 2 * TT],
                                  h_dram[:, hc, TT : 2 * TT])
            # h(tt2) rides the Activation HWDGE queue, which is idle after
            # the table loads until the first exp (~13us)
            for hc in range(8):
                nc.scalar.dma_start(h_sb[:, hc, 2 * TT : 3 * TT],
                                    h_dram[:, hc, 2 * TT : 3 * TT])
            for hc in range(8):
                nc.sync.dma_start(h_sb[:, hc, 3 * TT : S],
                                  h_dram[:, hc, 3 * TT : S])
            # second halves of the RoPE tables ride the idle Pool SWDGE
            # queue (land ~10us, needed from ~13us on)
            nc.gpsimd.dma_start(cos2_sb[:, TT:S], cos2d[:, TT:S])
            nc.gpsimd.dma_start(sin2_sb[:, TT:S], sin2d[:, TT:S])
            nc.gpsimd.dma_start(sin_sb[:, TT:S], sind[:, TT:S])
            nc.gpsimd.dma_start(wqr_sb[:, :, 128:256],
                                wqr_dram[:, :, 128:256])
            for oc in range(2):
                nc.sync.dma_start(wo_sb[:, oc, :], wo_dram[:, oc, :])

            ident = pp.tile([128, 128], BF, tag="ident")
            make_identity(nc, ident[:])
            # preload the exp table set while input DMAs stream
            warm = pp.tile([1, 8], F32, tag="warm")
            nc.any.memset(warm[:], 0.0)
            nc.scalar.activation(warm[:], warm[:], AF.Exp)

            # roped q, 2 heads per tile (head 2p at rows 0:64, 2p+1 at 64:128)
            qrot = [pp.tile([128, S], BF, tag=f"qrot{p}", name=f"qrot{p}") for p in range(2)]
            # roped k duplicated on both partition halves (matmul requires
            # lhsT/rhs base partitions to match; BIR forbids cross-base
            # SBUF/SBUF elementwise reads, so the halves are materialized)
            k2 = pp.tile([128, S], BF, tag="k2")
            # V with ones column for the fused softmax denominator
            vaug = pp.tile([128, KC, 65], BF, tag="vaug")
            nc.any.memset(vaug[:], 1.0)
            # normalized attention output in [d, token] layout per pair
            anorm = [pp.tile([128, S], BF, tag=f"anorm{o}", name=f"anorm{o}") for o in range(2)]
            # per-unit reciprocal denominators (unit = pair + 2*qt)
            rec_sb = pp.tile([128, 8, 8], F32, tag="rec_sb")

            def rope64(ps_blk, dest, tts, tmp_tag):
                """RoPE one 64-row head block [64, TT] read directly from
                PSUM (PSUM-input elementwise ops may mix partition bases;
                SBUF/SBUF may not). The bf16 t1+rt add runs in DVE 2x."""
                t1 = rp.tile([64, TT], BF, tag=f"t1{tmp_tag}")
                rt = rp.tile([64, TT], BF, tag=f"rt{tmp_tag}")
                nc.vector.tensor_tensor(t1[:], ps_blk[0:64, :], cos_sb[:, tts], MULT)
                nc.vector.tensor_tensor(
                    rt[0:32, :], ps_blk[32:64, :], sin_sb[0:32, tts], MULT
                )
                nc.vector.tensor_tensor(
                    rt[32:64, :], ps_blk[0:32, :], sin_sb[32:64, tts], MULT
                )
                nc.vector.tensor_tensor(dest, t1[:], rt[:], ADD)

            with (
                tc.tile_pool(name="psP", bufs=2, space="PSUM") as psP,
                tc.tile_pool(name="psS", bufs=2, space="PSUM") as psS,
                tc.tile_pool(name="psACC", bufs=1, space="PSUM") as psA,
            ):
                def proj_finish(rc, tt, ps, on_act=False):
                    tts = bass.ts(tt, TT)
                    if rc == 2:
                        rope64(ps[0:64, :], k2[0:64, tts], tts, "k")
                        nc.gpsimd.tensor_copy(k2[64:128, tts], k2[0:64, tts])
                    else:
                        rope64(ps[0:64, :], qrot[rc][0:64, tts], tts, "qa")
                        rope64(ps[64:128, :], qrot[rc][64:128, tts], tts, "qb")

                def wqk_cols(rc):
                    # q pairs at 0:128 / 128:256; k-only at 256:320
                    return slice(256, 320) if rc == 2 else bass.ts(rc, 128)

                def proj_chunk(rc, tt):
                    """One projection tile: 8 accumulating matmuls, then RoPE
                    (rc<2: q pair [128,TT]; rc==2: k rows only [64,TT])."""
                    tts = bass.ts(tt, TT)
                    rows = 64 if rc == 2 else 128
                    ps = psP.tile([rows, TT], F32, tag="proj",
                                  name=f"proj{rc}_{tt}")
                    for hc in range(8):
                        nc.tensor.matmul(
                            ps[:],
                            wqk_sb[:, hc, wqk_cols(rc)],
                            h_sb[:, hc, tts],
                            start=(hc == 0),
                            stop=(hc == 7),
                        )
                    proj_finish(rc, tt, ps)

                def kproj_rot(tt):
                    """k projection + rotate-half projection; k2 rows 0:64
                    via three full-width DVE ops, then the partition-half
                    duplicate for the head-B score lhsT."""
                    tts = bass.ts(tt, TT)
                    ps = psP.tile([64, TT], F32, tag="proj", name=f"kp{tt}")
                    ps2 = psP.tile([64, TT], F32, tag="proj", name=f"kr{tt}")
                    for hc in range(8):
                        nc.tensor.matmul(
                            ps[:], wqk_sb[:, hc, 256:320], h_sb[:, hc, tts],
                            start=(hc == 0), stop=(hc == 7),
                        )
                    for hc in range(8):
                        nc.tensor.matmul(
                            ps2[:], wqr_sb[:, hc, 256:320], h_sb[:, hc, tts],
                            start=(hc == 0), stop=(hc == 7),
                        )
                    t1 = rp.tile([64, TT], BF, tag="kt1")
                    t2 = rp.tile([64, TT], BF, tag="kt2")
                    nc.vector.tensor_tensor(t1[:], ps[:], cos2_sb[0:64, tts], MULT)
                    nc.vector.tensor_tensor(t2[:], ps2[:], sin2_sb[0:64, tts], MULT)
                    nc.vector.tensor_tensor(k2[0:64, tts], t1[:], t2[:], ADD)
                    nc.gpsimd.tensor_copy(k2[64:128, tts], k2[0:64, tts])

                def add_kproj_rot_filler(tt):
                    tts = bass.ts(tt, TT)
                    cell = {}

                    def mk(which, h0):
                        def go():
                            if which == 0 and h0 == 0:
                                cell["ps"] = psP.tile(
                                    [64, TT], F32, tag="proj", name=f"kp{tt}")
                                cell["ps2"] = psP.tile(
                                    [64, TT], F32, tag="proj", name=f"kr{tt}")
                            w = wqk_sb if which == 0 else wqr_sb
                            ps = cell["ps"] if which == 0 else cell["ps2"]
                            for hc in (h0, h0 + 1):
                                nc.tensor.matmul(
                                    ps[:], w[:, hc, 256:320],
                                    h_sb[:, hc, tts],
                                    start=(hc == 0), stop=(hc == 7),
                                )
                            if which == 1 and h0 == 6:
                                t1 = rp.tile([64, TT], BF, tag="kt1")
                                t2 = rp.tile([64, TT], BF, tag="kt2")
                                nc.vector.tensor_tensor(
                                    t1[:], cell["ps"][:],
                                    cos2_sb[0:64, tts], MULT)
                                nc.vector.tensor_tensor(
                                    t2[:], cell["ps2"][:],
                                    sin2_sb[0:64, tts], MULT)
                                nc.vector.tensor_tensor(
                                    k2[0:64, tts], t1[:], t2[:], ADD)
                                nc.gpsimd.tensor_copy(
                                    k2[64:128, tts], k2[0:64, tts])
                        return go

                    for which in (0, 1):
                        for h0 in (0, 2, 4, 6):
                            filler.append(mk(which, h0))

                def qproj_rot(rc, tt):
                    """q projection with the rotate-half permutation folded
                    into a second (host-permuted) weight matrix: qrot =
                    q*cos + rot(q)*sin in three full-width DVE ops."""
                    tts = bass.ts(tt, TT)
                    ps = psP.tile([128, TT], F32, tag="proj",
                                  name=f"qp{rc}_{tt}")
                    ps2 = psP.tile([128, TT], F32, tag="proj",
                                   name=f"qr{rc}_{tt}")
                    for hc in range(8):
                        nc.tensor.matmul(
                            ps[:], wqk_sb[:, hc, bass.ts(rc, 128)],
                            h_sb[:, hc, tts],
                            start=(hc == 0), stop=(hc == 7),
                        )
                    for hc in range(8):
                        nc.tensor.matmul(
                            ps2[:], wqr_sb[:, hc, bass.ts(rc, 128)],
                            h_sb[:, hc, tts],
                            start=(hc == 0), stop=(hc == 7),
                        )
                    t1 = rp.tile([128, TT], BF, tag="rt1")
                    t2 = rp.tile([128, TT], BF, tag="rt2")
                    nc.vector.tensor_tensor(t1[:], ps[:], cos2_sb[:, tts], MULT)
                    nc.vector.tensor_tensor(t2[:], ps2[:], sin2_sb[:, tts], MULT)
                    nc.vector.tensor_tensor(qrot[rc][:, tts], t1[:], t2[:], ADD)

                def add_qproj_rot_filler(rc, tt):
                    tts = bass.ts(tt, TT)
                    cell = {}

                    def mk(which, h0):
                        def go():
                            if which == 0 and h0 == 0:
                                cell["ps"] = psP.tile(
                                    [128, TT], F32, tag="proj",
                                    name=f"qp{rc}_{tt}")
                                cell["ps2"] = psP.tile(
                                    [128, TT], F32, tag="proj",
                                    name=f"qr{rc}_{tt}")
                            w = wqk_sb if which == 0 else wqr_sb
                            ps = cell["ps"] if which == 0 else cell["ps2"]
                            for hc in (h0, h0 + 1):
                                nc.tensor.matmul(
                                    ps[:], w[:, hc, bass.ts(rc, 128)],
                                    h_sb[:, hc, tts],
                                    start=(hc == 0), stop=(hc == 7),
                                )
                            if which == 1 and h0 == 6:
                                t1 = rp.tile([128, TT], BF, tag="rt1")
                                t2 = rp.tile([128, TT], BF, tag="rt2")
                                nc.vector.tensor_tensor(
                                    t1[:], cell["ps"][:], cos2_sb[:, tts], MULT)
                                nc.vector.tensor_tensor(
                                    t2[:], cell["ps2"][:], sin2_sb[:, tts], MULT)
                                nc.vector.tensor_tensor(
                                    qrot[rc][:, tts], t1[:], t2[:], ADD)
                        return go

                    for which in (0, 1):
                        for h0 in (0, 2, 4, 6):
                            filler.append(mk(which, h0))

                def vproj_one(c):
                    # V projection flipped to [token, d] orientation: writes
                    # the vaug [kpos, d] layout directly (no PE transpose)
                    tcs = bass.ts(c, 128)
                    vp = psP.tile([128, 64], F32, tag="proj", name=f"vp{c}")
                    for hc in range(8):
                        nc.tensor.matmul(
                            vp[:],
                            h_sb[:, hc, tcs],
                            wqk_sb[:, hc, 320:384],
                            start=(hc == 0),
                            stop=(hc == 7),
                        )
                    nc.vector.tensor_copy(vaug[:, c, 0:64], vp[:])

                # filler queue: zero-arg closures, each emitting ~1-2 PE
                # matmuls (plus attached vector/pool work)
                filler = []

                def emit_filler(n):
                    for _ in range(n):
                        if not filler:
                            return
                        filler.pop(0)()

                def add_proj_filler(rc, tt):
                    # split one proj chunk into 4 closures of 2 matmuls each
                    tts = bass.ts(tt, TT)
                    cell = {}

                    def mk(h0):
                        def go():
                            if h0 == 0:
                                cell["ps"] = psP.tile(
                                    [64 if rc == 2 else 128, TT], F32,
                                    tag="proj", name=f"proj{rc}_{tt}",
                                )
                            ps = cell["ps"]
                            for hc in (h0, h0 + 1):
                                nc.tensor.matmul(
                                    ps[:],
                                    wqk_sb[:, hc, wqk_cols(rc)],
                                    h_sb[:, hc, tts],
                                    start=(hc == 0),
                                    stop=(hc == 7),
                                )
                            if h0 == 6:
                                proj_finish(rc, tt, ps)
                        return go

                    for h0 in (0, 2, 4, 6):
                        filler.append(mk(h0))

                def add_vproj_filler(tt):
                    for c in range(4 * tt, 4 * tt + 4):
                        filler.append(lambda c=c: vproj_one(c))

                def outproj_one(tch, ht, ob_on_act=False, dma_q="sp",
                                pool=None, ptag="proj"):
                    tcs = bass.ts(tch, 128)
                    hts = bass.ts(ht, TT)
                    po = (pool or psP).tile([128, TT], F32, tag=ptag,
                                            name=f"po{tch}_{ht}")
                    for oc in range(2):
                        nc.tensor.matmul(
                            po[:],
                            anorm[oc][:, tcs],
                            wo_sb[:, oc, hts],
                            start=(oc == 0),
                            stop=(oc == 1),
                        )
                    ob = op_.tile([128, TT], F32, tag="ob")
                    if ob_on_act:
                        nc.scalar.activation(ob[:], po[:], AF.Copy)
                    else:
                        nc.vector.tensor_copy(ob[:], po[:])
                    eng = {"sp": nc.sync, "act": nc.scalar,
                           "pool": nc.gpsimd}[dma_q]
                    eng.dma_start(out[tcs, hts], ob[:])

                def add_outproj_filler(qt):
                    for tch in range(4 * qt, 4 * qt + 4):
                        for ht in range(2):
                            filler.append(
                                lambda tch=tch, ht=ht: outproj_one(tch, ht))

                # deferred per-unit finalize -------------------------------
                # normTS: DVE-only (recips + tensor_scalar into anq tiles);
                # frees the psA slot.  transpose+copy drain via fillers.
                pending = []   # at most one (norm_ts, transp_one, pair, qt)
                last_pacc = [None]

                def make_finalize(pair, qt, pacc):
                    uid = 2 * qt + pair
                    anqs = []

                    def norm_ts():
                        for half in range(2):
                            nc.vector.reciprocal_approx_fast(
                                rec_sb[:, uid, 4 * half : 4 * half + 4],
                                pacc[:, 512 * half + 64 : 512 * half + 324 : 65],
                            )
                        for qc in range(4):
                            anq = np_.tile([128, 128], BF, tag="anq")
                            anqs.append(anq)
                            for hl in range(2):
                                g = hl * 4 + qc
                                nc.vector.tensor_scalar_mul(
                                    anq[:, 64 * hl : 64 * hl + 64],
                                    pacc[:, GOFF[g] : GOFF[g] + 64],
                                    rec_sb[:, uid, g : g + 1],
                                )

                    def transp_one(qc):
                        ptr = psP.tile([128, 128], BF, tag="proj",
                                       name=f"at{pair}_{qt}_{qc}")
                        nc.tensor.transpose(ptr[:], anqs[qc][:], ident[:])
                        nc.vector.tensor_copy(
                            anorm[pair][:, qt * TT + 128 * qc :
                                        qt * TT + 128 * qc + 128],
                            ptr[:],
                        )

                    return norm_ts, transp_one

                def run_pending():
                    if not pending:
                        return
                    norm_ts, transp_one, pair, qt = pending.pop()
                    norm_ts()
                    for qc in range(4):
                        filler.append(lambda qc=qc, t=transp_one: t(qc))
                    if pair == 1:
                        add_outproj_filler(qt)

                I32 = mybir.dt.int32

                def sc_exp(pair, qt, c, on_dve=False):
                    cs = bass.ts(c, 128)
                    qts = bass.ts(qt, TT)
                    sc2 = psS.tile([128, 1024], F32, tag="sc",
                                   name=f"sc{pair}_{qt}_{c}")
                    nc.tensor.matmul(
                        sc2[:, 0:TT], k2[0:64, cs], qrot[pair][0:64, qts],
                        start=True, stop=True,
                    )
                    nc.tensor.matmul(
                        sc2[:, TT:1024], k2[64:128, cs],
                        qrot[pair][64:128, qts],
                        start=True, stop=True,
                    )
                    ex = ep.tile([128, 1024], BF, tag="exp")
                    if on_dve:
                        # Schraudolph exp on the vector engine: build the f32
                        # bit pattern of e^x as int32(x * 2^23/ln2 + bias),
                        # then reinterpret and round to bf16.  Offloads ~1
                        # exp per unit from the saturated Activation engine.
                        exi = ep.tile([128, 1024], I32, tag="expi", bufs=2)
                        nc.vector.tensor_scalar(
                            exi[:], sc2[:], 12102203.161561485, 1064986316.0,
                            MULT, ADD,
                        )
                        nc.gpsimd.tensor_copy(ex[:], exi[:].bitcast(F32))
                    else:
                        nc.scalar.activation(ex[:], sc2[:], AF.Exp)
                    return ex

                def pv(pacc, ex, c):
                    for g in range(8):
                        hl, qc = divmod(g, 4)
                        # one accumulation group per 2KB PSUM zero region
                        # (bank): only the first group in a bank starts it,
                        # only the last stops it; the other groups' first
                        # writes land on pending-zero bytes and overwrite.
                        nc.tensor.matmul(
                            pacc[:, GOFF[g] : GOFF[g] + 65],
                            ex[:, hl * TT + qc * 128 :
                               hl * TT + qc * 128 + 128],
                            vaug[:, c, :],
                            start=(c == 0 and g in (0, 4)),
                            stop=(c == KC - 1 and g in (3, 7)),
                        )

                def attention_unit(pair, qt, fill_per_kc=2, pre_block=None):
                    """Scores+exp+PV for 2 heads (pair) x 512 queries (qt).

                    Scores are emitted two k-chunks ahead of the PV/filler
                    work so the Activation engine always has a ~2-exp runway
                    against transient PE stalls (psS double-buffering paces
                    the PE to the exp stream automatically)."""
                    if pre_block is not None:
                        pre_block(0)
                    dve_kc = () if pre_block is not None else (14,)
                    exs = [sc_exp(pair, qt, 0), sc_exp(pair, qt, 1)]
                    run_pending()
                    pacc = psA.tile([128, 1024], F32, tag="att",
                                    name=f"att{pair}_{qt}")
                    exs.append(sc_exp(pair, qt, 2))
                    for c in range(KC):
                        if pre_block is not None and c % 4 == 0 and c > 0:
                            pre_block(c // 4)
                        if c + 3 < KC:
                            exs.append(sc_exp(pair, qt, c + 3,
                                              on_dve=(c + 3) in dve_kc))
                        pv(pacc, exs[c], c)
                        emit_filler(fill_per_kc)
                    pending.append(make_finalize(pair, qt, pacc) + (pair, qt))
                    last_pacc[0] = pacc

                # ---------- emission schedule -----------------------------
                # startup: k/v for tt0, q-pair0 for tt0
                proj_chunk(2, 0)
                qproj_rot(0, 0)
                # tt0 V chunks inline: they only need h(tt0) and fill the PE
                # while RoPE runs; must be emitted before PV(kc0)
                for c in range(4):
                    vproj_one(c)
                # k(tt1) inline too: the PE is otherwise idle until the tt0
                # q-RoPE lands, and its own RoPE then rides right behind on
                # the DVE queue, a full block ahead of the kc4 deadline
                proj_chunk(2, 1)

                # unit (0,0): weave remaining k/v chunks a block ahead of
                # the kc block that needs them
                def pre_block_00(b):
                    if b == 0:
                        add_proj_filler(2, 2)
                        add_vproj_filler(1)
                    elif b == 1:
                        add_proj_filler(2, 3)
                        add_vproj_filler(2)
                    elif b == 2:
                        add_vproj_filler(3)
                        add_qproj_rot_filler(1, 0)

                attention_unit(0, 0, fill_per_kc=3, pre_block=pre_block_00)

                # remaining units; queue proj fillers so qrot[p][tt] is
                # ready one unit ahead
                add_proj_filler(0, 1)
                add_proj_filler(1, 1)
                attention_unit(1, 0)
                add_proj_filler(0, 2)
                attention_unit(0, 1, fill_per_kc=1)
                add_proj_filler(1, 2)
                attention_unit(1, 1, fill_per_kc=1)
                add_proj_filler(0, 3)
                attention_unit(0, 2, fill_per_kc=1)
                add_proj_filler(1, 3)
                attention_unit(1, 2, fill_per_kc=1)
                attention_unit(0, 3, fill_per_kc=1)
                po_pre = {}

                def pre_tail(b):
                    if b == 3:
                        for ht in range(2):
                            po = psP.tile([128, TT], F32, tag="proj",
                                          name=f"po12_{ht}")
                            nc.tensor.matmul(
                                po[:], anorm[0][:, bass.ts(12, 128)],
                                wo_sb[:, 0, bass.ts(ht, TT)],
                                start=True, stop=False,
                            )
                            po_pre[ht] = po

                attention_unit(1, 3, fill_per_kc=2, pre_block=pre_tail)

                # ---------- tail: finalize last unit, outproj(3) ----------
                _, _, lpair, lqt = pending.pop()
                luid = 2 * lqt + lpair
                lpacc = last_pacc[0]
                emit_filler(len(filler))
                for half in range(2):
                    nc.vector.reciprocal_approx_fast(
                        rec_sb[:, luid, 4 * half : 4 * half + 4],
                        lpacc[:, 512 * half + 64 : 512 * half + 324 : 65],
                    )
                for qc in range(4):
                    anq = np_.tile([128, 128], BF, tag="anq")
                    for hl in range(2):
                        g = hl * 4 + qc
                        nc.vector.tensor_scalar_mul(
                            anq[:, 64 * hl : 64 * hl + 64],
                            lpacc[:, GOFF[g] : GOFF[g] + 64],
                            rec_sb[:, luid, g : g + 1],
                        )
                    ptr = psS.tile([128, 128], BF, tag="sc",
                                   name=f"at_t_{qc}")
                    nc.tensor.transpose(ptr[:], anq[:], ident[:])
                    nc.vector.tensor_copy(
                        anorm[lpair][:, lqt * TT + 128 * qc :
                                     lqt * TT + 128 * qc + 128],
                        ptr[:],
                    )
                    tch = 4 * lqt + qc
                    if qc == 0:
                        # tch12 pair-0 half was accumulated during (1,3)
                        for ht in range(2):
                            po = po_pre[ht]
                            hts = bass.ts(ht, TT)
                            nc.tensor.matmul(
                                po[:], anorm[1][:, bass.ts(12, 128)],
                                wo_sb[:, 1, hts],
                                start=False, stop=True,
                            )
                            ob = op_.tile([128, TT], F32, tag="ob")
                            nc.scalar.activation(ob[:], po[:], AF.Copy)
                            eng = nc.sync if ht == 0 else nc.gpsimd
                            eng.dma_start(out[bass.ts(12, 128), hts], ob[:])
                    else:
                        outproj_one(tch, 0, ob_on_act=True, dma_q="sp",
                                    pool=psS, ptag="sc")
                        outproj_one(tch, 1, ob_on_act=False,
                                    dma_q=("pool" if qc < 3 else "act"),
                                    pool=psP)
    nc.finalize()
    return nc


def _get_nc():
    global _nc_cache
    if _nc_cache is None:
        _nc_cache = _build_bass()
    return _nc_cache


def _shard_inputs(hidden_states, cos, sin, w_qkv, w_o):
    """Build per-core input maps. Core c = (b = c // 4, g = c % 4)."""
    cosT = np.ascontiguousarray(cos.T.astype(np.float32))         # [64, S]
    sinT = sin.T.astype(np.float32)
    sinmod = np.concatenate([-sinT[0:32], sinT[32:64]], axis=0)    # sign folded
    sinmod = np.ascontiguousarray(sinmod).astype(_BF16)
    cos2 = np.ascontiguousarray(np.concatenate([cosT, cosT], 0)).astype(_BF16)
    sin2 = np.ascontiguousarray(np.concatenate([sinT, sinT], 0)).astype(_BF16)

    hT = [
        np.ascontiguousarray(hidden_states[b].T).astype(_BF16) for b in range(B)
    ]
    in_maps = []
    for c in range(NCORES):
        b, g = divmod(c, 4)
        q_rows = w_qkv[256 * g : 256 * g + 256] * SCALE
        # rotate-half permuted+negated q rows: row i<32 of each 64-block
        # becomes -row(i+32), row i>=32 becomes +row(i-32)
        qr = q_rows.reshape(4, 2, 32, HID)
        q_rot = np.concatenate([-qr[:, 1], qr[:, 0]], axis=1)      # [4,2,32,H]
        q_rot = q_rot.reshape(256, HID)
        k_rows = w_qkv[1024 + 64 * g : 1024 + 64 * g + 64]
        k_rot = np.concatenate([-k_rows[32:64], k_rows[0:32]], axis=0)
        v_rows = w_qkv[1280 + 64 * g : 1280 + 64 * g + 64]
        wqk = np.concatenate([q_rows, k_rows, v_rows], axis=0)     # [384, 1024]
        wqkT = np.ascontiguousarray(wqk.T).astype(_BF16)           # [1024, 384]
        woT = np.ascontiguousarray(
            w_o[:, 256 * g : 256 * g + 256].T
        ).astype(_BF16)                                            # [256, 1024]
        wqr = np.concatenate([q_rot, k_rot], axis=0)               # [320, 1024]
        wqrT = np.ascontiguousarray(wqr.T).astype(_BF16)           # [1024, 320]
        in_maps.append(
            {
                "hT": hT[b],
                "wqkT": wqkT,
                "wqrd": wqrT,
                "woT": woT,
                "cos2d": cos2,
                "sin2d": sin2,
                "sind": sinmod,
            }
        )
    return in_maps


def _run(inputs, **spmd_kwargs):
    from concourse.bass_utils import run_bass_kernel_spmd

    nc = _get_nc()
    in_maps = _shard_inputs(**inputs)
    res = run_bass_kernel_spmd(
        nc, in_maps, core_ids=list(range(NCORES)), **spmd_kwargs
    )
    outs = []
    for b in range(B):
        acc = res.results[4 * b]["out"].astype(np.float32).copy()
        for g in range(1, 4):
            acc += res.results[4 * b + g]["out"]
        outs.append(acc)
    return np.stack(outs, axis=0), res


def kernel(**inputs):
    out, _ = _run(inputs)
    return out
